# revision 6
# baseline (speedup 1.0000x reference)
"""Trainium2 Bass kernel for a GPT-style transformer block (B=8, T=1024, C=1024, 16 heads).

Strategy: data-parallel over the batch dimension — one batch element per
NeuronCore, full weights broadcast to every core, no collectives.

Per-core layout: activations are kept feature-major ("transposed world") for
the matmuls; layernorm + residuals run token-major; the two layernorm outputs
are transposed on the PE. Attention probabilities are computed in bf16 and
transposed with the DMA xbar (hardware 16x128 transpose), everything else runs
through the PE in fp32r (full-rate fp32).
"""

import sys

if "/opt/trn_rl_repo" not in sys.path:
    sys.path.insert(0, "/opt/trn_rl_repo")

import numpy as np

B, T, C = 8, 1024, 1024
NH, HD = 16, 64
C3, C4 = 3 * C, 4 * C
EPS = 1e-5
NCORES = 8
NT = T // 128          # token tiles
NKC = C // 128         # feature chunks
NPAIR = NH // 2        # head pairs (two heads packed per 128 partitions)
TBAND = 512            # attention / mlp T-band
MASK_VAL = -1e9

_cache = {}


def _build():
    import concourse.bass as bass
    import concourse.mybir as mybir
    import concourse.tile as tile
    from concourse import bacc
    from concourse.masks import make_causal_mask, make_identity

    dt = mybir.dt
    f32, f32r, bf16 = dt.float32, dt.float32r, dt.bfloat16
    AF = mybir.ActivationFunctionType
    Alu = mybir.AluOpType

    nc = bacc.Bacc(
        "TRN2",
        target_bir_lowering=False,
        debug=False,
        enable_asserts=True,
        num_devices=NCORES,
        num_swdge_queues=4,
    )

    x_d = nc.dram_tensor("x", [T, C], f32, kind="ExternalInput")
    wqkv_d = nc.dram_tensor("wqkv", [C, C3], f32r, kind="ExternalInput")
    bqkv_d = nc.dram_tensor("bqkv", [C3], f32, kind="ExternalInput")
    wap_d = nc.dram_tensor("wap", [C, C], f32r, kind="ExternalInput")
    bap_d = nc.dram_tensor("bap", [C], f32, kind="ExternalInput")
    wfc_d = nc.dram_tensor("wfc", [C, C4], f32r, kind="ExternalInput")
    bfc_d = nc.dram_tensor("bfc", [C4], f32, kind="ExternalInput")
    wmp_d = nc.dram_tensor("wmp", [C4, C], f32r, kind="ExternalInput")
    bmp_d = nc.dram_tensor("bmp", [C], f32, kind="ExternalInput")
    y_d = nc.dram_tensor("y", [T, C], f32, kind="ExternalOutput")
    yaT_d = nc.dram_tensor("yaT_scratch", [C, T], f32r)  # attention-out bounce

    def bcast_ap(vec_ap, parts=128):
        return bass.AP(
            tensor=vec_ap.tensor,
            offset=vec_ap.offset,
            ap=[[0, parts]] + list(vec_ap.ap),
        )

    with tile.TileContext(nc) as tc:
        import contextlib

        est = contextlib.ExitStack()
        with est:
            const = est.enter_context(tc.tile_pool(name="const", bufs=1))
            small = est.enter_context(tc.tile_pool(name="small", bufs=4))
            y1p = est.enter_context(tc.tile_pool(name="y1", bufs=1))

            ident = const.tile([128, 128], f32)
            make_identity(nc, ident[:])
            mask = const.tile([128, 128], f32)
            make_causal_mask(nc, mask[:], mask_val=MASK_VAL)
            eps_t = const.tile([128, 1], f32)
            nc.vector.memset(eps_t[:], EPS)
            bqkv_sb = const.tile([128, 24], f32)
            nc.gpsimd.dma_start(bqkv_sb[:], bqkv_d[:].rearrange("(m p) -> p m", p=128))
            bfc_sb = const.tile([128, 32], f32)
            nc.gpsimd.dma_start(bfc_sb[:], bfc_d[:].rearrange("(m p) -> p m", p=128))
            bap_bc = const.tile([128, C], f32)
            nc.gpsimd.dma_start(bap_bc[:], bcast_ap(bap_d[:]))
            bmp_bc = const.tile([128, C], f32)
            nc.gpsimd.dma_start(bmp_bc[:], bcast_ap(bmp_d[:]))

            y1 = y1p.tile([128, NT * C], f32)  # token-major, tile t at cols [t*C,(t+1)*C)

            def layernorm(x_ap, out_ap):
                # x_ap/out_ap: [128, C] token-major
                stats = small.tile([128, 2, 6], f32, tag="lnstats")
                xr = x_ap.rearrange("p (s d) -> p s d", d=512)
                for s in range(2):
                    nc.vector.bn_stats(out=stats[:, s, :], in_=xr[:, s, :])
                mv = small.tile([128, 2], f32, tag="lnmv")
                nc.vector.bn_aggr(out=mv[:], in_=stats[:])
                rstd = small.tile([128, 1], f32, tag="lnrstd")
                nc.scalar.activation(rstd[:], mv[:, 1:2], AF.Sqrt, bias=eps_t[:])
                nc.vector.reciprocal(rstd[:], rstd[:])
                nc.vector.tensor_scalar(
                    out=out_ap, in0=x_ap, scalar1=mv[:, 0:1], scalar2=rstd[:],
                    op0=Alu.subtract, op1=Alu.mult,
                )

            with tc.tile_pool(name="xp", bufs=1) as xp:
                x_sb = xp.tile([128, NT * C], f32)

                # ---------------- Phase A+B: LN1, transpose, QKV ----------------
                with tc.tile_pool(name="qkT", bufs=1) as qkTp, \
                     tc.tile_pool(name="vnat", bufs=1) as vnatp:
                    qkT = qkTp.tile([128, 16 * T], bf16)   # Q^T,K^T: m-tile m at cols [m*T,(m+1)*T)
                    vnat = vnatp.tile([128, NT * C], bf16)  # V token-major: tile t at [t*C,(t+1)*C)

                    with tc.tile_pool(name="xh1T", bufs=1) as xh1Tp, \
                         tc.tile_pool(name="xh1", bufs=2) as xh1p, \
                         tc.tile_pool(name="vT", bufs=2) as vTp, \
                         tc.tile_pool(name="wstA", bufs=3) as wstA, \
                         tc.tile_pool(name="psA", bufs=4, space="PSUM") as psA, \
                         tc.tile_pool(name="psB", bufs=4, space="PSUM") as psB:
                        xh1T = xh1Tp.tile([128, NKC * T], f32r)  # chunk kc at cols [kc*T,(kc+1)*T)

                        for t in range(NT):
                            nc.gpsimd.dma_start(
                                x_sb[:, t * C:(t + 1) * C], x_d[t * 128:(t + 1) * 128, :]
                            )
                            xh1 = xh1p.tile([128, C], f32, tag="xh1")
                            layernorm(x_sb[:, t * C:(t + 1) * C], xh1[:])
                            for kc in range(NKC):
                                pst = psA.tile([128, 128], f32, tag="pst")
                                nc.tensor.transpose(
                                    pst[:],
                                    xh1[:, kc * 128:(kc + 1) * 128],
                                    ident[:],
                                )
                                nc.vector.tensor_copy(
                                    xh1T[:, kc * T + t * 128: kc * T + t * 128 + 128], pst[:]
                                )

                        # QKV^T: out m-tile rows = 3C features; lhsT = wqkv col-block
                        for m in range(24):
                            wblk = wstA.tile([128, NKC * 128], f32r, tag="wblk")
                            nc.gpsimd.dma_start(
                                wblk[:].rearrange("p (kc n) -> p kc n", n=128),
                                wqkv_d[:, m * 128:(m + 1) * 128].rearrange(
                                    "(kc p) n -> p kc n", p=128
                                ),
                            )
                            if m < 16:
                                dst = qkT[:, m * T:(m + 1) * T]
                            else:
                                vT = vTp.tile([128, T], bf16, tag="vT")
                                dst = vT[:]
                            for n in range(T // 512):
                                ps = psB.tile([128, 512], f32, tag="psB")
                                for kc in range(NKC):
                                    nc.tensor.matmul(
                                        ps[:],
                                        wblk[:, kc * 128:(kc + 1) * 128],
                                        xh1T[:, kc * T + n * 512: kc * T + (n + 1) * 512],
                                        start=(kc == 0),
                                        stop=(kc == NKC - 1),
                                    )
                                nc.vector.tensor_scalar(
                                    out=dst[:, n * 512:(n + 1) * 512], in0=ps[:],
                                    scalar1=bqkv_sb[:, m:m + 1], scalar2=None,
                                    op0=Alu.add,
                                )
                            if m >= 16:
                                cv = m - 16  # feature chunk of V
                                for t in range(NT):
                                    nc.sync.dma_start(
                                        vnat[:, t * C + cv * 128: t * C + cv * 128 + 128],
                                        vT[:, t * 128:(t + 1) * 128],
                                        transpose=True,
                                    )

                    # ---------------- Phase C: attention ----------------
                    with tc.tile_pool(name="ptp", bufs=1) as ptp, \
                         tc.tile_pool(name="pp", bufs=3) as pp, \
                         tc.tile_pool(name="psS", bufs=4, space="PSUM") as psS, \
                         tc.tile_pool(name="psO", bufs=2, space="PSUM") as psO:
                        for p in range(NPAIR):
                            # P^T band buffers: [Tk-chunk][128, TBAND] per head
                            pts = [[ptp.tile([128, TBAND], bf16, tag=f"pt{h}_{j}", name=f"pt{h}_{j}")
                                    for j in range(NT)] for h in range(2)]
                            for band in range(T // TBAND):
                                for ii in range(TBAND // 128):
                                    ti = band * (TBAND // 128) + ii
                                    W = 128 * (ti + 1)
                                    for h in range(2):
                                        base = 64 * h
                                        ph = pp.tile([128, T], bf16, tag=f"p{h}")
                                        lparts = []
                                        c0 = 0
                                        while c0 < W:
                                            cw = min(512, W - c0)
                                            ps = psS.tile([128, 512], f32, tag="psS")
                                            nc.tensor.matmul(
                                                ps[:, :cw],
                                                qkT[base:base + 64,
                                                    p * T + ti * 128: p * T + ti * 128 + 128],
                                                qkT[base:base + 64,
                                                    (8 + p) * T + c0: (8 + p) * T + c0 + cw],
                                                start=True, stop=True,
                                                tile_position=(base, 0),
                                            )
                                            if c0 + cw == W:
                                                nc.vector.tensor_tensor(
                                                    out=ps[:, cw - 128:cw],
                                                    in0=ps[:, cw - 128:cw],
                                                    in1=mask[:], op=Alu.add,
                                                )
                                            lp = small.tile([128, 1], f32, tag="lp")
                                            nc.scalar.activation(
                                                ph[:, c0:c0 + cw], ps[:, :cw], AF.Exp,
                                                scale=0.125, accum_out=lp[:],
                                            )
                                            lparts.append(lp)
                                            c0 += cw
                                        ltot = lparts[0]
                                        for lp in lparts[1:]:
                                            nc.vector.tensor_add(ltot[:], ltot[:], lp[:])
                                        nc.vector.reciprocal(ltot[:], ltot[:])
                                        nc.vector.tensor_scalar_mul(
                                            ph[:, :W], ph[:, :W], ltot[:]
                                        )
                                        for j in range(ti + 1):
                                            nc.sync.dma_start(
                                                pts[h][j][:, ii * 128:(ii + 1) * 128],
                                                ph[:, j * 128:(j + 1) * 128],
                                                transpose=True,
                                            )
                                # PV for this band
                                jmax = band * (TBAND // 128) + (TBAND // 128) - 1
                                pso = psO.tile([128, TBAND], f32, tag="psO")
                                for j in range(jmax + 1):
                                    off = max(0, j * 128 - band * TBAND)
                                    for h in range(2):
                                        nc.tensor.matmul(
                                            pso[64 * h:64 * h + 64, off:TBAND],
                                            vnat[:, j * C + p * 128 + 64 * h:
                                                 j * C + p * 128 + 64 * h + 64],
                                            pts[h][j][:, off:TBAND],
                                            start=(j == 0), stop=(j == jmax),
                                            tile_position=(0, 64 * h),
                                            skip_group_check=True,
                                        )
                                yb = pp.tile([128, TBAND], f32r, tag="yband")
                                nc.vector.tensor_copy(yb[:], pso[:])
                                nc.gpsimd.dma_start(
                                    yaT_d[p * 128:(p + 1) * 128,
                                          band * TBAND:(band + 1) * TBAND],
                                    yb[:],
                                )

                # ---------------- Phase D: aproj + residual + LN2 ----------------
                with tc.tile_pool(name="xh2T", bufs=1) as xh2Tp:
                    xh2T = xh2Tp.tile([128, NKC * T], f32r)
                    with tc.tile_pool(name="yaT", bufs=1) as yaTp, \
                         tc.tile_pool(name="wap", bufs=1) as wapp, \
                         tc.tile_pool(name="xh2", bufs=2) as xh2p, \
                         tc.tile_pool(name="psD", bufs=4, space="PSUM") as psD, \
                         tc.tile_pool(name="psDT", bufs=4, space="PSUM") as psDT:
                        yaT = yaTp.tile([128, NKC * T], f32r)
                        nc.gpsimd.dma_start(
                            yaT[:].rearrange("p (kc t) -> p kc t", kc=8),
                            yaT_d[:, :].rearrange("(kc p) t -> p kc t", p=128),
                        )
                        wap_sb = wapp.tile([128, NKC * C], f32r)
                        nc.gpsimd.dma_start(
                            wap_sb[:].rearrange("p (kc n) -> p kc n", kc=8),
                            wap_d[:, :].rearrange("(kc p) n -> p kc n", p=128),
                        )
                        for m in range(NT):
                            for n in range(C // 512):
                                ps = psD.tile([128, 512], f32, tag="psD")
                                for kc in range(NKC):
                                    nc.tensor.matmul(
                                        ps[:],
                                        yaT[:, kc * T + m * 128: kc * T + m * 128 + 128],
                                        wap_sb[:, kc * C + n * 512: kc * C + (n + 1) * 512],
                                        start=(kc == 0), stop=(kc == NKC - 1),
                                    )
                                sl = slice(m * C + n * 512, m * C + (n + 1) * 512)
                                nc.vector.tensor_add(y1[:, sl], ps[:], x_sb[:, sl])
                                nc.vector.tensor_add(
                                    y1[:, sl], y1[:, sl], bap_bc[:, n * 512:(n + 1) * 512]
                                )
                            xh2 = xh2p.tile([128, C], f32, tag="xh2")
                            layernorm(y1[:, m * C:(m + 1) * C], xh2[:])
                            for kc in range(NKC):
                                pst = psDT.tile([128, 128], f32, tag="psDT")
                                nc.tensor.transpose(
                                    pst[:],
                                    xh2[:, kc * 128:(kc + 1) * 128],
                                    ident[:],
                                )
                                nc.vector.tensor_copy(
                                    xh2T[:, kc * T + m * 128: kc * T + m * 128 + 128],
                                    pst[:],
                                )

                    # ---------------- Phase E: MLP ----------------
                    with tc.tile_pool(name="ht", bufs=1) as htp, \
                         tc.tile_pool(name="wstE", bufs=3) as wstE, \
                         tc.tile_pool(name="yout", bufs=3) as youtp:
                        for tch in range(T // TBAND):
                            ht = htp.tile([128, 32 * TBAND], f32r, tag="ht")
                            with tc.tile_pool(name=f"psFC{tch}", bufs=2, space="PSUM") as psFC:
                                for m in range(32):
                                    wblk = wstE.tile([128, NKC * 128], f32r, tag="wfcblk")
                                    nc.gpsimd.dma_start(
                                        wblk[:].rearrange("p (kc n) -> p kc n", n=128),
                                        wfc_d[:, m * 128:(m + 1) * 128].rearrange(
                                            "(kc p) n -> p kc n", p=128
                                        ),
                                    )
                                    ps = psFC.tile([128, 512], f32, tag="psFC")
                                    for kc in range(NKC):
                                        nc.tensor.matmul(
                                            ps[:],
                                            wblk[:, kc * 128:(kc + 1) * 128],
                                            xh2T[:, kc * T + tch * TBAND:
                                                 kc * T + (tch + 1) * TBAND],
                                            start=(kc == 0), stop=(kc == NKC - 1),
                                        )
                                    nc.scalar.activation(
                                        ht[:, m * TBAND:(m + 1) * TBAND], ps[:],
                                        AF.Gelu, bias=bfc_sb[:, m:m + 1],
                                    )
                            with tc.tile_pool(name=f"psMP{tch}", bufs=1, space="PSUM") as psMP:
                                psy = [[psMP.tile([128, 512], f32, tag=f"psy{mt}_{n}", name=f"psy{mt}_{n}")
                                        for n in range(2)] for mt in range(4)]
                                for kc in range(32):
                                    wblk = wstE.tile([128, C], f32r, tag="wmpblk")
                                    nc.gpsimd.dma_start(
                                        wblk[:], wmp_d[kc * 128:(kc + 1) * 128, :]
                                    )
                                    for mt in range(4):
                                        for n in range(2):
                                            nc.tensor.matmul(
                                                psy[mt][n][:],
                                                ht[:, kc * TBAND + mt * 128:
                                                   kc * TBAND + mt * 128 + 128],
                                                wblk[:, n * 512:(n + 1) * 512],
                                                start=(kc == 0), stop=(kc == 31),
                                                skip_group_check=True,
                                            )
                                for mt in range(4):
                                    t = tch * 4 + mt
                                    for n in range(2):
                                        yo = youtp.tile([128, 512], f32, tag="yout")
                                        sl = slice(t * C + n * 512, t * C + (n + 1) * 512)
                                        nc.vector.tensor_add(yo[:], psy[mt][n][:], y1[:, sl])
                                        nc.vector.tensor_add(
                                            yo[:], yo[:], bmp_bc[:, n * 512:(n + 1) * 512]
                                        )
                                        nc.gpsimd.dma_start(
                                            y_d[t * 128:(t + 1) * 128, n * 512:(n + 1) * 512],
                                            yo[:],
                                        )

    nc.finalize()
    return nc


def _get_nc():
    if "nc" not in _cache:
        _cache["nc"] = _build()
    return _cache["nc"]


def _prep_inputs(inputs):
    f32 = np.float32
    x = np.ascontiguousarray(np.asarray(inputs["x"], f32))
    ln1_g = np.asarray(inputs["ln1_g"], f32)
    ln1_b = np.asarray(inputs["ln1_b"], f32)
    ln2_g = np.asarray(inputs["ln2_g"], f32)
    ln2_b = np.asarray(inputs["ln2_b"], f32)
    w_attn = np.asarray(inputs["w_attn"], f32)
    b_attn = np.asarray(inputs["b_attn"], f32)
    wqkv = np.ascontiguousarray(ln1_g[:, None] * w_attn)
    bqkv = np.ascontiguousarray(b_attn + ln1_b @ w_attn)
    w_fc = np.asarray(inputs["w_fc"], f32)
    b_fc = np.asarray(inputs["b_fc"], f32)
    wfc = np.ascontiguousarray(ln2_g[:, None] * w_fc)
    bfc = np.ascontiguousarray(b_fc + ln2_b @ w_fc)
    shared = {
        "wqkv": wqkv,
        "bqkv": bqkv,
        "wap": np.ascontiguousarray(np.asarray(inputs["w_aproj"], f32)),
        "bap": np.ascontiguousarray(np.asarray(inputs["b_aproj"], f32)),
        "wfc": wfc,
        "bfc": bfc,
        "wmp": np.ascontiguousarray(np.asarray(inputs["w_mproj"], f32)),
        "bmp": np.ascontiguousarray(np.asarray(inputs["b_mproj"], f32)),
    }
    return x, shared


def kernel(**inputs):
    x, shared = _prep_inputs(inputs)
    nc = _get_nc()
    from concourse.bass_utils import run_bass_kernel_spmd

    in_maps = [dict(shared, x=x[i]) for i in range(NCORES)]
    res = run_bass_kernel_spmd(nc, in_maps, list(range(NCORES)))
    out = np.stack([res.results[i]["y"] for i in range(NCORES)], axis=0)
    return out.astype(np.float32)


if __name__ == "__main__":
    nc = _get_nc()
    print("built ok")


# revision 7
# speedup vs baseline: 115.3203x; 115.3203x over previous
"""Trainium2 Bass kernel for a GPT-style transformer block (B=8, T=1024, C=1024, 16 heads).

Strategy: data-parallel over the batch dimension — one batch element per
NeuronCore, full weights broadcast to every core, no collectives.

Per-core layout: activations are kept feature-major ("transposed world") for
the matmuls; layernorm + residuals run token-major; the two layernorm outputs
are transposed on the PE. Attention probabilities are computed in bf16 and
transposed with the DMA xbar (hardware 16x128 transpose), everything else runs
through the PE in fp32r (full-rate fp32).
"""

import sys

if "/opt/trn_rl_repo" not in sys.path:
    sys.path.insert(0, "/opt/trn_rl_repo")

import numpy as np

B, T, C = 8, 1024, 1024
NH, HD = 16, 64
C3, C4 = 3 * C, 4 * C
EPS = 1e-5
NCORES = 8
NT = T // 128          # token tiles
NKC = C // 128         # feature chunks
NPAIR = NH // 2        # head pairs (two heads packed per 128 partitions)
TBAND = 512            # attention / mlp T-band
MASK_VAL = -1e9

_cache = {}


def _build():
    import concourse.bass as bass
    import concourse.mybir as mybir
    import concourse.tile as tile
    from concourse import bacc
    from concourse.masks import make_causal_mask, make_identity

    dt = mybir.dt
    f32, f32r, bf16 = dt.float32, dt.float32r, dt.bfloat16
    AF = mybir.ActivationFunctionType
    Alu = mybir.AluOpType

    nc = bacc.Bacc(
        "TRN2",
        target_bir_lowering=False,
        debug=False,
        enable_asserts=True,
        num_devices=NCORES,
        num_swdge_queues=4,
    )

    x_d = nc.dram_tensor("x", [T, C], f32, kind="ExternalInput")
    wqkv_d = nc.dram_tensor("wqkv", [C, C3], f32r, kind="ExternalInput")
    bqkv_d = nc.dram_tensor("bqkv", [C3], f32, kind="ExternalInput")
    wap_d = nc.dram_tensor("wap", [C, C], f32r, kind="ExternalInput")
    bap_d = nc.dram_tensor("bap", [C], f32, kind="ExternalInput")
    wfc_d = nc.dram_tensor("wfc", [C, C4], f32r, kind="ExternalInput")
    bfc_d = nc.dram_tensor("bfc", [C4], f32, kind="ExternalInput")
    wmp_d = nc.dram_tensor("wmp", [C4, C], f32r, kind="ExternalInput")
    bmp_d = nc.dram_tensor("bmp", [C], f32, kind="ExternalInput")
    y_d = nc.dram_tensor("y", [T, C], f32, kind="ExternalOutput")
    yaT_d = nc.dram_tensor("yaT_scratch", [C, T], f32r)  # attention-out bounce

    def bcast_ap(vec_ap, parts=128):
        return bass.AP(
            tensor=vec_ap.tensor,
            offset=vec_ap.offset,
            ap=[[0, parts]] + list(vec_ap.ap),
        )

    with tile.TileContext(nc) as tc:
        import contextlib

        est = contextlib.ExitStack()
        with est:
            const = est.enter_context(tc.tile_pool(name="const", bufs=1))
            small = est.enter_context(tc.tile_pool(name="small", bufs=4))
            y1p = est.enter_context(tc.tile_pool(name="y1", bufs=1))

            ident = const.tile([128, 128], f32)
            make_identity(nc, ident[:])
            mask = const.tile([128, 128], f32)
            make_causal_mask(nc, mask[:], mask_val=MASK_VAL)
            eps_t = const.tile([128, 1], f32)
            nc.vector.memset(eps_t[:], EPS)
            bqkv_sb = const.tile([128, 24], f32)
            nc.gpsimd.dma_start(bqkv_sb[:], bqkv_d[:].rearrange("(m p) -> p m", p=128))
            bfc_sb = const.tile([128, 32], f32)
            nc.gpsimd.dma_start(bfc_sb[:], bfc_d[:].rearrange("(m p) -> p m", p=128))
            bap_bc = const.tile([128, C], f32)
            nc.gpsimd.dma_start(bap_bc[:], bcast_ap(bap_d[:]))
            bmp_bc = const.tile([128, C], f32)
            nc.gpsimd.dma_start(bmp_bc[:], bcast_ap(bmp_d[:]))

            y1 = y1p.tile([128, NT * C], f32)  # token-major, tile t at cols [t*C,(t+1)*C)

            def layernorm(x_ap, out_ap):
                # x_ap/out_ap: [128, C] token-major
                stats = small.tile([128, 2, 6], f32, tag="lnstats")
                xr = x_ap.rearrange("p (s d) -> p s d", d=512)
                for s in range(2):
                    nc.vector.bn_stats(out=stats[:, s, :], in_=xr[:, s, :])
                mv = small.tile([128, 2], f32, tag="lnmv")
                nc.vector.bn_aggr(out=mv[:], in_=stats[:])
                rstd = small.tile([128, 1], f32, tag="lnrstd")
                nc.scalar.activation(rstd[:], mv[:, 1:2], AF.Sqrt, bias=eps_t[:])
                nc.vector.reciprocal(rstd[:], rstd[:])
                nc.vector.tensor_scalar(
                    out=out_ap, in0=x_ap, scalar1=mv[:, 0:1], scalar2=rstd[:],
                    op0=Alu.subtract, op1=Alu.mult,
                )

            with tc.tile_pool(name="xp", bufs=1) as xp:
                x_sb = xp.tile([128, NT * C], f32)

                # ---------------- Phase A+B: LN1, transpose, QKV ----------------
                with tc.tile_pool(name="qkT", bufs=1) as qkTp, \
                     tc.tile_pool(name="vnat", bufs=1) as vnatp:
                    qkT = qkTp.tile([128, 16 * T], bf16)   # Q^T,K^T: m-tile m at cols [m*T,(m+1)*T)
                    vnat = vnatp.tile([128, NT * C], bf16)  # V token-major: tile t at [t*C,(t+1)*C)

                    with tc.tile_pool(name="xh1T", bufs=1) as xh1Tp, \
                         tc.tile_pool(name="xh1", bufs=2) as xh1p, \
                         tc.tile_pool(name="vT", bufs=2) as vTp, \
                         tc.tile_pool(name="wstA", bufs=3) as wstA, \
                         tc.tile_pool(name="psA", bufs=4, space="PSUM") as psA, \
                         tc.tile_pool(name="psB", bufs=4, space="PSUM") as psB:
                        xh1T = xh1Tp.tile([128, NKC * T], f32r)  # chunk kc at cols [kc*T,(kc+1)*T)

                        for t in range(NT):
                            nc.gpsimd.dma_start(
                                x_sb[:, t * C:(t + 1) * C], x_d[t * 128:(t + 1) * 128, :]
                            )
                            xh1 = xh1p.tile([128, C], f32, tag="xh1")
                            layernorm(x_sb[:, t * C:(t + 1) * C], xh1[:])
                            for kc in range(NKC):
                                pst = psA.tile([128, 128], f32, tag="pst")
                                nc.tensor.transpose(
                                    pst[:],
                                    xh1[:, kc * 128:(kc + 1) * 128],
                                    ident[:],
                                )
                                nc.vector.tensor_copy(
                                    xh1T[:, kc * T + t * 128: kc * T + t * 128 + 128], pst[:]
                                )

                        # QKV^T: out m-tile rows = 3C features; lhsT = wqkv col-block
                        for m in range(24):
                            wblk = wstA.tile([128, NKC * 128], f32r, tag="wblk")
                            nc.gpsimd.dma_start(
                                wblk[:].rearrange("p (kc n) -> p kc n", n=128),
                                wqkv_d[:, m * 128:(m + 1) * 128].rearrange(
                                    "(kc p) n -> p kc n", p=128
                                ),
                            )
                            if m < 16:
                                dst = qkT[:, m * T:(m + 1) * T]
                            else:
                                vT = vTp.tile([128, T], bf16, tag="vT")
                                dst = vT[:]
                            for n in range(T // 512):
                                ps = psB.tile([128, 512], f32, tag="psB")
                                for kc in range(NKC):
                                    nc.tensor.matmul(
                                        ps[:],
                                        wblk[:, kc * 128:(kc + 1) * 128],
                                        xh1T[:, kc * T + n * 512: kc * T + (n + 1) * 512],
                                        start=(kc == 0),
                                        stop=(kc == NKC - 1),
                                    )
                                nc.vector.tensor_scalar(
                                    out=dst[:, n * 512:(n + 1) * 512], in0=ps[:],
                                    scalar1=bqkv_sb[:, m:m + 1], scalar2=None,
                                    op0=Alu.add,
                                )
                            if m >= 16:
                                cv = m - 16  # feature chunk of V
                                for t in range(NT):
                                    nc.sync.dma_start(
                                        vnat[:, t * C + cv * 128: t * C + cv * 128 + 128],
                                        vT[:, t * 128:(t + 1) * 128],
                                        transpose=True,
                                    )

                    # ---------------- Phase C: attention ----------------
                    with tc.tile_pool(name="ptp", bufs=1) as ptp, \
                         tc.tile_pool(name="pp", bufs=3) as pp, \
                         tc.tile_pool(name="psS", bufs=4, space="PSUM") as psS, \
                         tc.tile_pool(name="psO", bufs=2, space="PSUM") as psO:
                        for p in range(NPAIR):
                            # P^T band buffers: [Tk-chunk][128, TBAND] per head
                            pts = [[ptp.tile([128, TBAND], bf16, tag=f"pt{h}_{j}", name=f"pt{h}_{j}")
                                    for j in range(NT)] for h in range(2)]
                            for band in range(T // TBAND):
                                for ii in range(TBAND // 128):
                                    ti = band * (TBAND // 128) + ii
                                    W = 128 * (ti + 1)
                                    for h in range(2):
                                        base = 64 * h
                                        ph = pp.tile([128, T], bf16, tag=f"p{h}")
                                        lparts = []
                                        c0 = 0
                                        while c0 < W:
                                            cw = min(512, W - c0)
                                            ps = psS.tile([128, 512], f32, tag="psS")
                                            nc.tensor.matmul(
                                                ps[:, :cw],
                                                qkT[base:base + 64,
                                                    p * T + ti * 128: p * T + ti * 128 + 128],
                                                qkT[base:base + 64,
                                                    (8 + p) * T + c0: (8 + p) * T + c0 + cw],
                                                start=True, stop=True,
                                                tile_position=(base, 0),
                                            )
                                            if c0 + cw == W:
                                                nc.vector.tensor_tensor(
                                                    out=ps[:, cw - 128:cw],
                                                    in0=ps[:, cw - 128:cw],
                                                    in1=mask[:], op=Alu.add,
                                                )
                                            lp = small.tile([128, 1], f32, tag="lp")
                                            nc.scalar.activation(
                                                ph[:, c0:c0 + cw], ps[:, :cw], AF.Exp,
                                                scale=0.125, accum_out=lp[:],
                                            )
                                            lparts.append(lp)
                                            c0 += cw
                                        ltot = lparts[0]
                                        for lp in lparts[1:]:
                                            nc.vector.tensor_add(ltot[:], ltot[:], lp[:])
                                        nc.vector.reciprocal(ltot[:], ltot[:])
                                        nc.vector.tensor_scalar_mul(
                                            ph[:, :W], ph[:, :W], ltot[:]
                                        )
                                        for j in range(ti + 1):
                                            nc.sync.dma_start(
                                                pts[h][j][:, ii * 128:(ii + 1) * 128],
                                                ph[:, j * 128:(j + 1) * 128],
                                                transpose=True,
                                            )
                                # PV for this band
                                jmax = band * (TBAND // 128) + (TBAND // 128) - 1
                                pso = psO.tile([128, TBAND], f32, tag="psO")
                                for j in range(jmax + 1):
                                    off = max(0, j * 128 - band * TBAND)
                                    for h in range(2):
                                        nc.tensor.matmul(
                                            pso[64 * h:64 * h + 64, off:TBAND],
                                            vnat[:, j * C + p * 128 + 64 * h:
                                                 j * C + p * 128 + 64 * h + 64],
                                            pts[h][j][:, off:TBAND],
                                            start=(j == 0), stop=(j == jmax),
                                            tile_position=(0, 64 * h),
                                            skip_group_check=True,
                                        )
                                yb = pp.tile([128, TBAND], f32r, tag="yband")
                                nc.vector.tensor_copy(yb[:], pso[:])
                                nc.gpsimd.dma_start(
                                    yaT_d[p * 128:(p + 1) * 128,
                                          band * TBAND:(band + 1) * TBAND],
                                    yb[:],
                                )

                # ---------------- Phase D: aproj + residual + LN2 ----------------
                with tc.tile_pool(name="xh2T", bufs=1) as xh2Tp:
                    xh2T = xh2Tp.tile([128, NKC * T], f32r)
                    with tc.tile_pool(name="yaT", bufs=1) as yaTp, \
                         tc.tile_pool(name="wap", bufs=1) as wapp, \
                         tc.tile_pool(name="xh2", bufs=2) as xh2p, \
                         tc.tile_pool(name="psD", bufs=4, space="PSUM") as psD, \
                         tc.tile_pool(name="psDT", bufs=4, space="PSUM") as psDT:
                        yaT = yaTp.tile([128, NKC * T], f32r)
                        nc.gpsimd.dma_start(
                            yaT[:].rearrange("p (kc t) -> p kc t", kc=8),
                            yaT_d[:, :].rearrange("(kc p) t -> p kc t", p=128),
                        )
                        wap_sb = wapp.tile([128, NKC * C], f32r)
                        nc.gpsimd.dma_start(
                            wap_sb[:].rearrange("p (kc n) -> p kc n", kc=8),
                            wap_d[:, :].rearrange("(kc p) n -> p kc n", p=128),
                        )
                        for m in range(NT):
                            for n in range(C // 512):
                                ps = psD.tile([128, 512], f32, tag="psD")
                                for kc in range(NKC):
                                    nc.tensor.matmul(
                                        ps[:],
                                        yaT[:, kc * T + m * 128: kc * T + m * 128 + 128],
                                        wap_sb[:, kc * C + n * 512: kc * C + (n + 1) * 512],
                                        start=(kc == 0), stop=(kc == NKC - 1),
                                    )
                                sl = slice(m * C + n * 512, m * C + (n + 1) * 512)
                                nc.vector.tensor_add(y1[:, sl], ps[:], x_sb[:, sl])
                                nc.vector.tensor_add(
                                    y1[:, sl], y1[:, sl], bap_bc[:, n * 512:(n + 1) * 512]
                                )
                            xh2 = xh2p.tile([128, C], f32, tag="xh2")
                            layernorm(y1[:, m * C:(m + 1) * C], xh2[:])
                            for kc in range(NKC):
                                pst = psDT.tile([128, 128], f32, tag="psDT")
                                nc.tensor.transpose(
                                    pst[:],
                                    xh2[:, kc * 128:(kc + 1) * 128],
                                    ident[:],
                                )
                                nc.vector.tensor_copy(
                                    xh2T[:, kc * T + m * 128: kc * T + m * 128 + 128],
                                    pst[:],
                                )

                    # ---------------- Phase E: MLP ----------------
                    with tc.tile_pool(name="ht", bufs=1) as htp, \
                         tc.tile_pool(name="wstE", bufs=3) as wstE, \
                         tc.tile_pool(name="yout", bufs=3) as youtp:
                        for tch in range(T // TBAND):
                            ht = htp.tile([128, 32 * TBAND], f32r, tag="ht")
                            with tc.tile_pool(name=f"psFC{tch}", bufs=2, space="PSUM") as psFC:
                                for m in range(32):
                                    wblk = wstE.tile([128, NKC * 128], f32r, tag="wfcblk")
                                    nc.gpsimd.dma_start(
                                        wblk[:].rearrange("p (kc n) -> p kc n", n=128),
                                        wfc_d[:, m * 128:(m + 1) * 128].rearrange(
                                            "(kc p) n -> p kc n", p=128
                                        ),
                                    )
                                    ps = psFC.tile([128, 512], f32, tag="psFC")
                                    for kc in range(NKC):
                                        nc.tensor.matmul(
                                            ps[:],
                                            wblk[:, kc * 128:(kc + 1) * 128],
                                            xh2T[:, kc * T + tch * TBAND:
                                                 kc * T + (tch + 1) * TBAND],
                                            start=(kc == 0), stop=(kc == NKC - 1),
                                        )
                                    nc.scalar.activation(
                                        ht[:, m * TBAND:(m + 1) * TBAND], ps[:],
                                        AF.Gelu, bias=bfc_sb[:, m:m + 1],
                                    )
                            with tc.tile_pool(name=f"psMP{tch}", bufs=1, space="PSUM") as psMP:
                                psy = [[psMP.tile([128, 512], f32, tag=f"psy{mt}_{n}", name=f"psy{mt}_{n}")
                                        for n in range(2)] for mt in range(4)]
                                for kc in range(32):
                                    wblk = wstE.tile([128, C], f32r, tag="wmpblk")
                                    nc.gpsimd.dma_start(
                                        wblk[:], wmp_d[kc * 128:(kc + 1) * 128, :]
                                    )
                                    for mt in range(4):
                                        for n in range(2):
                                            nc.tensor.matmul(
                                                psy[mt][n][:],
                                                ht[:, kc * TBAND + mt * 128:
                                                   kc * TBAND + mt * 128 + 128],
                                                wblk[:, n * 512:(n + 1) * 512],
                                                start=(kc == 0), stop=(kc == 31),
                                                skip_group_check=True,
                                            )
                                for mt in range(4):
                                    t = tch * 4 + mt
                                    for n in range(2):
                                        yo = youtp.tile([128, 512], f32, tag="yout")
                                        sl = slice(t * C + n * 512, t * C + (n + 1) * 512)
                                        nc.vector.tensor_add(yo[:], psy[mt][n][:], y1[:, sl])
                                        nc.vector.tensor_add(
                                            yo[:], yo[:], bmp_bc[:, n * 512:(n + 1) * 512]
                                        )
                                        nc.gpsimd.dma_start(
                                            y_d[t * 128:(t + 1) * 128, n * 512:(n + 1) * 512],
                                            yo[:],
                                        )

    nc.finalize()
    return nc


def _get_nc():
    if "nc" not in _cache:
        _cache["nc"] = _build()
    return _cache["nc"]


def _prep_inputs(inputs):
    f32 = np.float32
    x = np.ascontiguousarray(np.asarray(inputs["x"], f32))
    ln1_g = np.asarray(inputs["ln1_g"], f32)
    ln1_b = np.asarray(inputs["ln1_b"], f32)
    ln2_g = np.asarray(inputs["ln2_g"], f32)
    ln2_b = np.asarray(inputs["ln2_b"], f32)
    w_attn = np.asarray(inputs["w_attn"], f32)
    b_attn = np.asarray(inputs["b_attn"], f32)
    wqkv = np.ascontiguousarray(ln1_g[:, None] * w_attn)
    bqkv = np.ascontiguousarray(b_attn + ln1_b @ w_attn)
    w_fc = np.asarray(inputs["w_fc"], f32)
    b_fc = np.asarray(inputs["b_fc"], f32)
    wfc = np.ascontiguousarray(ln2_g[:, None] * w_fc)
    bfc = np.ascontiguousarray(b_fc + ln2_b @ w_fc)
    shared = {
        "wqkv": wqkv,
        "bqkv": bqkv,
        "wap": np.ascontiguousarray(np.asarray(inputs["w_aproj"], f32)),
        "bap": np.ascontiguousarray(np.asarray(inputs["b_aproj"], f32)),
        "wfc": wfc,
        "bfc": bfc,
        "wmp": np.ascontiguousarray(np.asarray(inputs["w_mproj"], f32)),
        "bmp": np.ascontiguousarray(np.asarray(inputs["b_mproj"], f32)),
    }
    return x, shared


def _get_runner():
    """Build the 8-core PJRT callable once and cache it.

    Mirrors bass2jax.run_bass_via_pjrt's multi-core path, but jits a single
    reusable function (no donation — the kernel writes every output element),
    so repeated calls skip retracing and can run on device-resident inputs.
    """
    if "runner" in _cache:
        return _cache["runner"]
    nc = _get_nc()
    import jax
    from concourse import bass2jax, mybir
    from jax.experimental.shard_map import shard_map
    from jax.sharding import Mesh, NamedSharding, PartitionSpec

    bass2jax.install_neuronx_cc_hook()
    partition_name = (
        nc.partition_id_tensor.name if nc.partition_id_tensor else None
    )
    in_names, out_names, out_avals, zero_shapes = [], [], [], []
    for alloc in nc.m.functions[0].allocations:
        if not isinstance(alloc, mybir.MemoryLocationSet):
            continue
        name = alloc.memorylocations[0].name
        if alloc.kind == "ExternalInput":
            if name != partition_name:
                in_names.append(name)
        elif alloc.kind == "ExternalOutput":
            out_names.append(name)
            shape = tuple(alloc.tensor_shape)
            npdt = mybir.dt.np(alloc.dtype)
            out_avals.append(jax.core.ShapedArray(shape, npdt))
            zero_shapes.append((shape, npdt))
    n_params = len(in_names)
    all_names = tuple(
        in_names + out_names + ([partition_name] if partition_name else [])
    )

    def _body(*args):
        operands = list(args)
        if partition_name:
            operands.append(bass2jax.partition_id_tensor())
        outs = bass2jax._bass_exec_p.bind(
            *operands,
            out_avals=tuple(out_avals),
            in_names=all_names,
            out_names=tuple(out_names),
            lowering_input_output_aliases=(),
            sim_require_finite=True,
            sim_require_nnan=True,
            nc=nc,
        )
        return tuple(outs)

    devices = jax.devices()[:NCORES]
    mesh = Mesh(np.asarray(devices), ("core",))
    n_outs = len(out_names)
    fn = jax.jit(
        shard_map(
            _body,
            mesh=mesh,
            in_specs=(PartitionSpec("core"),) * (n_params + n_outs),
            out_specs=(PartitionSpec("core"),) * n_outs,
            check_rep=False,
        ),
        keep_unused=True,
    )
    sharding = NamedSharding(mesh, PartitionSpec("core"))
    runner = {
        "fn": fn,
        "in_names": in_names,
        "out_names": out_names,
        "zero_shapes": zero_shapes,
        "sharding": sharding,
        "mesh": mesh,
    }
    _cache["runner"] = runner
    return runner


def _concat_inputs(x, shared, in_names):
    arrs = []
    for name in in_names:
        if name == "x":
            arrs.append(x.reshape(NCORES * T, C))
        else:
            v = shared[name]
            arrs.append(np.concatenate([v] * NCORES, axis=0))
    return arrs


def kernel(**inputs):
    x, shared = _prep_inputs(inputs)
    r = _get_runner()
    concat_ins = _concat_inputs(x, shared, r["in_names"])
    zeros = [
        np.zeros((NCORES * s[0],) + tuple(s[1:]), d) for s, d in r["zero_shapes"]
    ]
    outs = r["fn"](*concat_ins, *zeros)
    y = np.asarray(outs[r["out_names"].index("y")])
    return y.reshape(NCORES, T, C).astype(np.float32)


if __name__ == "__main__":
    nc = _get_nc()
    print("built ok")


# revision 26
# speedup vs baseline: 4241.0467x; 36.7762x over previous
"""Trainium2 Bass kernel for a GPT-style transformer block (B=8, T=1024, C=1024, 16 heads).

Strategy: data-parallel over the batch dimension — one batch element per
NeuronCore, full weights broadcast to every core, no collectives.

Per-core layout: activations are kept feature-major ("transposed world") for
the matmuls; layernorm + residuals run token-major; the two layernorm outputs
are transposed on the PE. Attention probabilities are computed in bf16 and
transposed with the DMA xbar (hardware 16x128 transpose); the MLP hidden
activations and W_mproj run in bf16; everything else goes through the PE in
fp32r (full-rate fp32).
"""

import sys

if "/opt/trn_rl_repo" not in sys.path:
    sys.path.insert(0, "/opt/trn_rl_repo")

import numpy as np

B, T, C = 8, 1024, 1024
NH, HD = 16, 64
C3, C4 = 3 * C, 4 * C
EPS = 1e-5
NCORES = 8
NT = T // 128          # token tiles
NKC = C // 128         # feature chunks
NPAIR = NH // 2        # head pairs (two heads packed per 128 partitions)
TBAND = 512            # attention band
MASK_VAL = -1e9

_cache = {}


def _build():
    import contextlib

    import concourse.bass as bass
    import concourse.mybir as mybir
    import concourse.tile as tile
    from concourse import bacc
    from concourse.masks import make_causal_mask, make_identity

    dt = mybir.dt
    f32, f32r, bf16 = dt.float32, dt.float32r, dt.bfloat16
    AF = mybir.ActivationFunctionType
    Alu = mybir.AluOpType

    nc = bacc.Bacc(
        "TRN2",
        target_bir_lowering=False,
        debug=False,
        enable_asserts=True,
        num_devices=NCORES,
        num_swdge_queues=4,
    )

    x_d = nc.dram_tensor("x", [T, C], f32, kind="ExternalInput")
    wqkv_d = nc.dram_tensor("wqkv", [24, 128, NKC * 128], f32r, kind="ExternalInput")
    bqkv_d = nc.dram_tensor("bqkv", [C3], f32, kind="ExternalInput")
    wap_d = nc.dram_tensor("wap", [C, C], f32r, kind="ExternalInput")
    bap_d = nc.dram_tensor("bap", [C], f32, kind="ExternalInput")
    wfc_d = nc.dram_tensor("wfc", [32, 128, NKC * 128], f32r, kind="ExternalInput")
    bfc_d = nc.dram_tensor("bfc", [C4], f32, kind="ExternalInput")
    wmp_d = nc.dram_tensor("wmp", [C4, C], f32r, kind="ExternalInput")
    bmp_d = nc.dram_tensor("bmp", [C], f32, kind="ExternalInput")
    y_d = nc.dram_tensor("y", [T, C], f32, kind="ExternalOutput")
    yaT_d = nc.dram_tensor("yaT_scratch", [C, T], f32r)  # attention-out bounce

    def bcast_ap(vec_ap, parts=128):
        return bass.AP(
            tensor=vec_ap.tensor,
            offset=vec_ap.offset,
            ap=[[0, parts]] + list(vec_ap.ap),
        )

    with tile.TileContext(nc) as tc:
        est = contextlib.ExitStack()
        with est:
            const = est.enter_context(tc.tile_pool(name="const", bufs=1))
            small = est.enter_context(tc.tile_pool(name="small", bufs=8))
            strp = est.enter_context(tc.tile_pool(name="stream", bufs=1))

            ident = const.tile([128, 128], f32)
            make_identity(nc, ident[:])
            mask = const.tile([128, 128], f32)
            make_causal_mask(nc, mask[:], mask_val=MASK_VAL)
            eps_t = const.tile([128, 1], f32)
            nc.vector.memset(eps_t[:], EPS)
            bqkv_sb = const.tile([128, 24], f32)
            nc.gpsimd.dma_start(bqkv_sb[:], bqkv_d[:].rearrange("(m p) -> p m", p=128))
            bfc_sb = const.tile([128, 32], f32)
            nc.gpsimd.dma_start(bfc_sb[:], bfc_d[:].rearrange("(m p) -> p m", p=128))
            bap_bc = const.tile([128, C], f32)
            nc.gpsimd.dma_start(bap_bc[:], bcast_ap(bap_d[:]))
            bmp_bc = const.tile([128, C], f32)
            nc.gpsimd.dma_start(bmp_bc[:], bcast_ap(bmp_d[:]))

            # residual stream: holds x, then y1 = x + attnproj (overwritten in place)
            stream = strp.tile([128, NT * C], f32)

            def layernorm(x_ap, out_ap):
                stats = small.tile([128, 2, 6], f32, tag="lnstats")
                xr = x_ap.rearrange("p (s d) -> p s d", d=512)
                for s in range(2):
                    nc.vector.bn_stats(out=stats[:, s, :], in_=xr[:, s, :])
                mv = small.tile([128, 2], f32, tag="lnmv")
                nc.vector.bn_aggr(out=mv[:], in_=stats[:])
                rstd = small.tile([128, 1], f32, tag="lnrstd")
                nc.scalar.activation(rstd[:], mv[:, 1:2], AF.Sqrt, bias=eps_t[:])
                nc.vector.reciprocal(rstd[:], rstd[:])
                nc.vector.tensor_scalar(
                    out=out_ap, in0=x_ap, scalar1=mv[:, 0:1], scalar2=rstd[:],
                    op0=Alu.subtract, op1=Alu.mult,
                )

            # W_aproj pool opened early; its DMA is emitted after the x loads
            d_scope = contextlib.ExitStack()
            wapp = d_scope.enter_context(tc.tile_pool(name="wap", bufs=1))
            wap_sb = wapp.tile([128, NKC * C], f32r)

            # ---------- Phase A+B+C: LN1, transpose, QKV + attention ----------
            with tc.tile_pool(name="xh1T", bufs=1) as xh1Tp, \
                 tc.tile_pool(name="xh1", bufs=2) as xh1p, \
                 tc.tile_pool(name="qkT", bufs=2) as qkTp, \
                 tc.tile_pool(name="vT", bufs=2) as vTp, \
                 tc.tile_pool(name="vnat", bufs=2) as vnatp, \
                 tc.tile_pool(name="wstA", bufs=3) as wstA, \
                 tc.tile_pool(name="ptp", bufs=1) as ptp, \
                 tc.tile_pool(name="pp", bufs=4) as pp, \
                 tc.tile_pool(name="psB", bufs=1, space="PSUM") as psB, \
                 tc.tile_pool(name="psS", bufs=4, space="PSUM") as psS, \
                 tc.tile_pool(name="psO", bufs=1, space="PSUM") as psO:
                xh1T = xh1Tp.tile([128, NKC * T], f32r)

                psA_scope = contextlib.ExitStack()
                psA = psA_scope.enter_context(
                    tc.tile_pool(name="psA", bufs=2, space="PSUM")
                )
                for t in range(NT):
                    nc.scalar.dma_start(
                        stream[:, t * C:(t + 1) * C], x_d[t * 128:(t + 1) * 128, :]
                    )
                    xh1 = xh1p.tile([128, C], f32, tag="xh1")
                    layernorm(stream[:, t * C:(t + 1) * C], xh1[:])
                    for kc in range(NKC):
                        pst = psA.tile([128, 128], f32, tag="pst")
                        nc.tensor.transpose(
                            pst[:], xh1[:, kc * 128:(kc + 1) * 128], ident[:]
                        )
                        nc.vector.tensor_copy(
                            xh1T[:, kc * T + t * 128: kc * T + t * 128 + 128], pst[:]
                        )

                psA_scope.close()
                nc.gpsimd.dma_start(
                    wap_sb[:].rearrange("p (kc n) -> p kc n", kc=8),
                    wap_d[:, :].rearrange("(kc p) n -> p kc n", p=128),
                )

                # ---- merged QKV + attention, software-pipelined by pair ----
                tiles = {}

                def emit_qkv(p):
                    qk = qkTp.tile([128, 2 * T], bf16, tag="qk", name="qk")
                    vT = vTp.tile([128, T], bf16, tag="vT", name="vT")
                    vn = vnatp.tile([128, NT * 128], bf16, tag="vn", name="vn")
                    tiles[p] = (qk, vn)
                    for mi, dst in ((p, qk[:, 0:T]), (8 + p, qk[:, T:2 * T]),
                                    (16 + p, vT[:])):
                        wblk = wstA.tile([128, NKC * 128], f32r, tag="wblk")
                        nc.gpsimd.dma_start(wblk[:], wqkv_d[mi])
                        for n in range(T // 512):
                            ps = psB.tile([128, 512], f32, tag="psB")
                            for kc in range(NKC):
                                nc.tensor.matmul(
                                    ps[:],
                                    wblk[:, kc * 128:(kc + 1) * 128],
                                    xh1T[:, kc * T + n * 512: kc * T + (n + 1) * 512],
                                    start=(kc == 0),
                                    stop=(kc == NKC - 1),
                                )
                            nc.vector.tensor_scalar(
                                out=dst[:, n * 512:(n + 1) * 512], in0=ps[:],
                                scalar1=bqkv_sb[:, mi:mi + 1], scalar2=None,
                                op0=Alu.add,
                            )
                    for t in range(NT):
                        nc.sync.dma_start(
                            vn[:, t * 128:(t + 1) * 128],
                            vT[:, t * 128:(t + 1) * 128],
                            transpose=True,
                        )

                def emit_s_phase(p, pts):
                    qk, _ = tiles[p]
                    for ti in range(NT):
                        W = 128 * (ti + 1)
                        band, ii = ti // (TBAND // 128), ti % (TBAND // 128)
                        bpts = pts[band]
                        for h in range(2):
                            base = 64 * h
                            ph = pp.tile([128, T], bf16, tag=f"p{h}", name=f"p{h}")
                            lparts = []
                            c0 = 0
                            while c0 < W:
                                cw = min(512, W - c0)
                                ps = psS.tile([128, 512], f32, tag="psS")
                                nc.tensor.matmul(
                                    ps[:, :cw],
                                    qk[base:base + 64, ti * 128: ti * 128 + 128],
                                    qk[base:base + 64, T + c0: T + c0 + cw],
                                    start=True, stop=True,
                                    tile_position=(base, 0),
                                )
                                if c0 + cw == W:
                                    nc.vector.tensor_tensor(
                                        out=ps[:, cw - 128:cw],
                                        in0=ps[:, cw - 128:cw],
                                        in1=mask[:], op=Alu.add,
                                    )
                                lp = small.tile([128, 1], f32, tag="lp", name="lp")
                                nc.scalar.activation(
                                    ph[:, c0:c0 + cw], ps[:, :cw], AF.Exp,
                                    scale=0.125, accum_out=lp[:],
                                )
                                lparts.append(lp)
                                c0 += cw
                            ltot = lparts[0]
                            for lp in lparts[1:]:
                                nc.vector.tensor_add(ltot[:], ltot[:], lp[:])
                            nc.vector.reciprocal(ltot[:], ltot[:])
                            nc.vector.tensor_scalar_mul(ph[:, :W], ph[:, :W], ltot[:])
                            for j in range(ti + 1):
                                nc.sync.dma_start(
                                    bpts[h][j][:, ii * 128:(ii + 1) * 128],
                                    ph[:, j * 128:(j + 1) * 128],
                                    transpose=True,
                                )

                def emit_pv(p, band, pts):
                    _, vn = tiles[p]
                    bpts = pts[band]
                    jmax = band * (TBAND // 128) + (TBAND // 128) - 1
                    pso = psO.tile([128, TBAND], f32, tag="psO", name="pso")
                    for j in range(jmax + 1):
                        off = max(0, j * 128 - band * TBAND)
                        for h in range(2):
                            nc.tensor.matmul(
                                pso[64 * h:64 * h + 64, off:TBAND],
                                vn[:, j * 128 + 64 * h: j * 128 + 64 * h + 64],
                                bpts[h][j][:, off:TBAND],
                                start=(j == 0), stop=(j == jmax),
                                tile_position=(0, 64 * h),
                                skip_group_check=True,
                            )
                    yb = pp.tile([128, TBAND], f32r, tag="yband", name="yb")
                    nc.vector.tensor_copy(yb[:], pso[:])
                    nc.gpsimd.dma_start(
                        yaT_d[p * 128:(p + 1) * 128,
                              band * TBAND:(band + 1) * TBAND],
                        yb[:],
                    )

                emit_qkv(0)
                for p in range(NPAIR):
                    pts = [[[ptp.tile([128, TBAND], bf16, tag=f"pt{b}_{h}_{j}",
                                      name=f"pt{b}_{h}_{j}")
                             for j in range((b + 1) * (TBAND // 128))]
                            for h in range(2)] for b in range(T // TBAND)]
                    emit_s_phase(p, pts)
                    if p + 1 < NPAIR:
                        emit_qkv(p + 1)
                    for band in range(T // TBAND):
                        emit_pv(p, band, pts)
                    del tiles[p]

            # ---------------- Phase D: aproj + residual + LN2 ----------------
            fc_scope = contextlib.ExitStack()
            xh2Tp = fc_scope.enter_context(
                tc.tile_pool(name="xh2T", bufs=1, side="right")
            )
            xh2T = xh2Tp.tile([128, NKC * T], f32r)
            yaTp = d_scope.enter_context(tc.tile_pool(name="yaT", bufs=1))
            yaT = yaTp.tile([128, NKC * T], f32r)
            for kc in range(NKC):
                nc.gpsimd.dma_start(
                    yaT[:, kc * T:(kc + 1) * T],
                    yaT_d[kc * 128:(kc + 1) * 128, :],
                )
            with tc.tile_pool(name="xh2", bufs=2) as xh2p, \
                 tc.tile_pool(name="psD", bufs=4, space="PSUM") as psD, \
                 tc.tile_pool(name="psDT", bufs=4, space="PSUM") as psDT:
                for m in range(NT):
                    for n in range(C // 512):
                        ps = psD.tile([128, 512], f32, tag="psD")
                        for kc in range(NKC):
                            nc.tensor.matmul(
                                ps[:],
                                yaT[:, kc * T + m * 128: kc * T + m * 128 + 128],
                                wap_sb[:, kc * C + n * 512: kc * C + (n + 1) * 512],
                                start=(kc == 0), stop=(kc == NKC - 1),
                            )
                        sl = slice(m * C + n * 512, m * C + (n + 1) * 512)
                        # y1 = x + attnproj + bias, in place over the stream
                        nc.vector.tensor_add(stream[:, sl], ps[:], stream[:, sl])
                        nc.vector.tensor_add(
                            stream[:, sl], stream[:, sl],
                            bap_bc[:, n * 512:(n + 1) * 512],
                        )
                    xh2 = xh2p.tile([128, C], f32, tag="xh2")
                    layernorm(stream[:, m * C:(m + 1) * C], xh2[:])
                    for kc in range(NKC):
                        pst = psDT.tile([128, 128], f32, tag="psDT")
                        nc.tensor.transpose(
                            pst[:], xh2[:, kc * 128:(kc + 1) * 128], ident[:]
                        )
                        nc.vector.tensor_copy(
                            xh2T[:, kc * T + m * 128: kc * T + m * 128 + 128],
                            pst[:],
                        )
            d_scope.close()

            # ---------------- Phase E: MLP (T-chunked, f32r) ----------------
            with tc.tile_pool(name="ht", bufs=1) as htp, \
                 tc.tile_pool(name="wstE", bufs=4) as wstE, \
                 tc.tile_pool(name="yout", bufs=3) as youtp:
                for tch in range(T // TBAND):
                    ht = htp.tile([128, 32 * TBAND], f32r, tag="ht", name="ht")
                    with tc.tile_pool(name=f"psFC{tch}", bufs=3, space="PSUM") as psFC:
                        for m in range(32):
                            wblk = wstE.tile([128, NKC * 128], f32r, tag="wfcblk")
                            nc.gpsimd.dma_start(wblk[:], wfc_d[m])
                            ps = psFC.tile([128, 512], f32, tag="psFC")
                            for kc in range(NKC):
                                nc.tensor.matmul(
                                    ps[:],
                                    wblk[:, kc * 128:(kc + 1) * 128],
                                    xh2T[:, kc * T + tch * TBAND:
                                         kc * T + (tch + 1) * TBAND],
                                    start=(kc == 0), stop=(kc == NKC - 1),
                                )
                            nc.scalar.activation(
                                ht[:, m * TBAND:(m + 1) * TBAND], ps[:],
                                AF.Gelu, bias=bfc_sb[:, m:m + 1],
                            )
                    with tc.tile_pool(name=f"psMP{tch}", bufs=1, space="PSUM") as psMP:
                        psy = [[psMP.tile([128, 512], f32, tag=f"psy{mt}_{n}",
                                          name=f"psy{mt}_{n}")
                                for n in range(2)] for mt in range(4)]
                        for kc in range(32):
                            wblk = wstE.tile([128, C], f32r, tag="wmpblk")
                            nc.gpsimd.dma_start(
                                wblk[:], wmp_d[kc * 128:(kc + 1) * 128, :]
                            )
                            for mt in range(4):
                                for n in range(2):
                                    nc.tensor.matmul(
                                        psy[mt][n][:],
                                        ht[:, kc * TBAND + mt * 128:
                                           kc * TBAND + mt * 128 + 128],
                                        wblk[:, n * 512:(n + 1) * 512],
                                        start=(kc == 0), stop=(kc == 31),
                                        skip_group_check=True,
                                    )
                        for mt in range(4):
                            t = tch * 4 + mt
                            for n in range(2):
                                yo = youtp.tile([128, 512], f32, tag="yout")
                                sl = slice(t * C + n * 512, t * C + (n + 1) * 512)
                                nc.vector.tensor_add(yo[:], psy[mt][n][:], stream[:, sl])
                                nc.vector.tensor_add(
                                    yo[:], yo[:], bmp_bc[:, n * 512:(n + 1) * 512]
                                )
                                nc.gpsimd.dma_start(
                                    y_d[t * 128:(t + 1) * 128, n * 512:(n + 1) * 512],
                                    yo[:],
                                )
                fc_scope.close()

    nc.finalize()
    return nc


def _get_nc():
    if "nc" not in _cache:
        _cache["nc"] = _build()
    return _cache["nc"]


def _prep_inputs(inputs):
    import ml_dtypes as _ml

    f32 = np.float32
    x = np.ascontiguousarray(np.asarray(inputs["x"], f32))
    ln1_g = np.asarray(inputs["ln1_g"], f32)
    ln1_b = np.asarray(inputs["ln1_b"], f32)
    ln2_g = np.asarray(inputs["ln2_g"], f32)
    ln2_b = np.asarray(inputs["ln2_b"], f32)
    w_attn = np.asarray(inputs["w_attn"], f32)
    b_attn = np.asarray(inputs["b_attn"], f32)
    wqkv = ln1_g[:, None] * w_attn
    wqkv = np.ascontiguousarray(
        wqkv.reshape(8, 128, 24, 128).transpose(2, 1, 0, 3).reshape(24, 128, 1024)
    )
    bqkv = np.ascontiguousarray(b_attn + ln1_b @ w_attn)
    w_fc = np.asarray(inputs["w_fc"], f32)
    b_fc = np.asarray(inputs["b_fc"], f32)
    wfc = ln2_g[:, None] * w_fc
    wfc = np.ascontiguousarray(
        wfc.reshape(8, 128, 32, 128).transpose(2, 1, 0, 3).reshape(32, 128, 1024)
    )
    bfc = np.ascontiguousarray(b_fc + ln2_b @ w_fc)
    shared = {
        "wqkv": wqkv,
        "bqkv": bqkv,
        "wap": np.ascontiguousarray(np.asarray(inputs["w_aproj"], f32)),
        "bap": np.ascontiguousarray(np.asarray(inputs["b_aproj"], f32)),
        "wfc": wfc,
        "bfc": bfc,
        "wmp": np.ascontiguousarray(np.asarray(inputs["w_mproj"], f32)),
        "bmp": np.ascontiguousarray(np.asarray(inputs["b_mproj"], f32)),
    }
    return x, shared


def _get_runner():
    """Build the 8-core PJRT callable once and cache it.

    Mirrors bass2jax.run_bass_via_pjrt's multi-core path, but jits a single
    reusable function (no donation — the kernel writes every output element),
    so repeated calls skip retracing and can run on device-resident inputs.
    """
    if "runner" in _cache:
        return _cache["runner"]
    nc = _get_nc()
    import jax
    from concourse import bass2jax, mybir
    from jax.experimental.shard_map import shard_map
    from jax.sharding import Mesh, NamedSharding, PartitionSpec

    bass2jax.install_neuronx_cc_hook()
    partition_name = (
        nc.partition_id_tensor.name if nc.partition_id_tensor else None
    )
    in_names, out_names, out_avals, zero_shapes = [], [], [], []
    for alloc in nc.m.functions[0].allocations:
        if not isinstance(alloc, mybir.MemoryLocationSet):
            continue
        name = alloc.memorylocations[0].name
        if alloc.kind == "ExternalInput":
            if name != partition_name:
                in_names.append(name)
        elif alloc.kind == "ExternalOutput":
            out_names.append(name)
            shape = tuple(alloc.tensor_shape)
            npdt = mybir.dt.np(alloc.dtype)
            out_avals.append(jax.core.ShapedArray(shape, npdt))
            zero_shapes.append((shape, npdt))
    n_params = len(in_names)
    all_names = tuple(
        in_names + out_names + ([partition_name] if partition_name else [])
    )

    def _body(*args):
        operands = list(args)
        if partition_name:
            operands.append(bass2jax.partition_id_tensor())
        outs = bass2jax._bass_exec_p.bind(
            *operands,
            out_avals=tuple(out_avals),
            in_names=all_names,
            out_names=tuple(out_names),
            lowering_input_output_aliases=(),
            sim_require_finite=True,
            sim_require_nnan=True,
            nc=nc,
        )
        return tuple(outs)

    devices = jax.devices()[:NCORES]
    mesh = Mesh(np.asarray(devices), ("core",))
    n_outs = len(out_names)
    fn = jax.jit(
        shard_map(
            _body,
            mesh=mesh,
            in_specs=(PartitionSpec("core"),) * (n_params + n_outs),
            out_specs=(PartitionSpec("core"),) * n_outs,
            check_rep=False,
        ),
        keep_unused=True,
    )
    sharding = NamedSharding(mesh, PartitionSpec("core"))
    runner = {
        "fn": fn,
        "in_names": in_names,
        "out_names": out_names,
        "zero_shapes": zero_shapes,
        "sharding": sharding,
        "mesh": mesh,
    }
    _cache["runner"] = runner
    return runner


def _concat_inputs(x, shared, in_names):
    arrs = []
    for name in in_names:
        if name == "x":
            arrs.append(x.reshape(NCORES * T, C))
        else:
            v = shared[name]
            arrs.append(np.concatenate([v] * NCORES, axis=0))
    return arrs


def _get_repeat_fn(k):
    """Jit K back-to-back executions of the NEFF, serialized by chaining each
    run's outputs into the next run's (unread) output-buffer operands."""
    key = ("repeat", k)
    if key in _cache:
        return _cache[key]
    nc = _get_nc()
    import jax
    from concourse import bass2jax
    from jax.experimental.shard_map import shard_map
    from jax.sharding import Mesh, PartitionSpec

    r = _get_runner()
    partition_name = (
        nc.partition_id_tensor.name if nc.partition_id_tensor else None
    )
    out_avals = [jax.core.ShapedArray(tuple(s), d) for s, d in r["zero_shapes"]]
    all_names = tuple(
        r["in_names"] + r["out_names"] + ([partition_name] if partition_name else [])
    )
    n_params = len(r["in_names"])

    def _body(*args):
        args = list(args)
        outs = None
        for _ in range(k):
            operands = list(args)
            if partition_name:
                operands.append(bass2jax.partition_id_tensor())
            outs = bass2jax._bass_exec_p.bind(
                *operands,
                out_avals=tuple(out_avals),
                in_names=all_names,
                out_names=tuple(r["out_names"]),
                lowering_input_output_aliases=(),
                sim_require_finite=True,
                sim_require_nnan=True,
                nc=nc,
            )
            for i in range(len(outs)):
                args[n_params + i] = outs[i]
        return tuple(outs)

    n_in = len(r["in_names"]) + len(r["out_names"])
    fn = jax.jit(
        shard_map(
            _body,
            mesh=r["mesh"],
            in_specs=(PartitionSpec("core"),) * n_in,
            out_specs=(PartitionSpec("core"),) * len(r["out_names"]),
            check_rep=False,
        ),
        keep_unused=True,
    )
    _cache[key] = fn
    return fn


def bench_repeat(x, shared, k_lo=8, k_hi=72, iters=8):
    """Estimate ns per kernel execution from the slope of K asynchronously
    queued executions (dispatch overlaps; device serializes the runs)."""
    import time

    import jax

    r = _get_runner()
    concat_ins = _concat_inputs(x, shared, r["in_names"])
    zeros = [
        np.zeros((NCORES * s[0],) + tuple(s[1:]), d) for s, d in r["zero_shapes"]
    ]
    dev = [jax.device_put(a, r["sharding"]) for a in concat_ins + zeros]
    jax.block_until_ready(dev)
    fn = r["fn"]
    outs = fn(*dev)
    jax.block_until_ready(outs)
    res = {}
    for k in (k_lo, k_hi):
        best = float("inf")
        for _ in range(iters):
            t0 = time.perf_counter()
            last = None
            for _i in range(k):
                last = fn(*dev)
            jax.block_until_ready(last)
            best = min(best, time.perf_counter() - t0)
        res[k] = best
        print(f"  K={k}: min wall {best*1e3:.1f} ms")
    return (res[k_hi] - res[k_lo]) / (k_hi - k_lo) * 1e9


def kernel(**inputs):
    x, shared = _prep_inputs(inputs)
    r = _get_runner()
    concat_ins = _concat_inputs(x, shared, r["in_names"])
    zeros = [
        np.zeros((NCORES * s[0],) + tuple(s[1:]), d) for s, d in r["zero_shapes"]
    ]
    outs = r["fn"](*concat_ins, *zeros)
    y = np.asarray(outs[r["out_names"].index("y")])
    return y.reshape(NCORES, T, C).astype(np.float32)


if __name__ == "__main__":
    nc = _get_nc()
    print("built ok")
